# revision 9
# baseline (speedup 1.0000x reference)
"""GCNAlign 2-layer GCN forward on 8 trn2 NeuronCores — v3.

Same architecture as the baseline kernel (balanced target tiles, fp16
dma_gather of source rows, one-hot scatter matmuls on the PE, AllGather of
the x tables between layers), with three efficiency changes driven by
hardware traces:

 1. Gathers are merged: one dma_gather covers a GROUP of 8 target tiles
    (per table half), amortizing the ~0.9us fixed Q7 cost per call
    (392 calls -> ~104) at the price of padding each tile's edge count to
    a multiple of 128.
 2. One-hot matrices are built with a single DVE tensor_tensor is_equal
    against a free-axis-broadcast target column (290ns/[128,128]) instead
    of the 1.8us tensor_scalar path + cast.  Padding slots use a poison
    target (999) so their rows are all-zero and gather-buffer garbage is
    harmless (no buffer memsets needed).
 3. The edge weight ew = 1/in_degree(target) is constant per target, so it
    is removed from the one-hots and applied as a per-partition scale when
    evacuating the PSUM accumulator (fused into the existing copy).

Collectives are interleaved with the other branch's gather stream so the
GpSimd engine (the critical path: ~10.6ns/gathered edge) stays busy.
"""

import os
import time
import heapq
import numpy as np

import concourse.bass as bass
import concourse.bacc as bacc
import concourse.mybir as mybir
from concourse.tile import TileContext
from concourse import bass_utils

F32 = mybir.dt.float32
F16 = mybir.dt.float16
I32 = mybir.dt.int32
I16 = mybir.dt.int16

N_NODES = 50000
N_EDGES = 800000
DIM = 200
N_CORES = 8
NPC = N_NODES // N_CORES  # 6250 nodes per core
DPAD = 256
POISON = 999.0
GRP = 4                   # tiles merged per gather call


def tile_sizes_for(npc):
    sizes = [128] * (npc // 128)
    if npc % 128:
        sizes.append(npc % 128)
    return sizes


# ---------------------------------------------------------------------------
# Host-side planning
# ---------------------------------------------------------------------------

def plan_branch(edges, ew, n_nodes, n_cores, sizes):
    """Balanced node->tile assignment (as baseline) + per-tile edge lists."""
    src = np.asarray(edges[0], dtype=np.int64)
    tgt = np.asarray(edges[1], dtype=np.int64)
    ew = np.asarray(ew, dtype=np.float32).reshape(-1)
    T = len(sizes)
    n_tiles = n_cores * T
    caps = np.tile(np.asarray(sizes, dtype=np.int64), n_cores)
    assert caps.sum() == n_nodes

    deg = np.bincount(tgt, minlength=n_nodes)
    order = np.argsort(-deg, kind="stable")
    heap = [(0, t) for t in range(n_tiles)]
    heapq.heapify(heap)
    remaining = caps.copy()
    tile_of_node = np.empty(n_nodes, dtype=np.int32)
    tile_members = [[] for _ in range(n_tiles)]
    for node in order:
        while True:
            s, t = heapq.heappop(heap)
            if remaining[t] > 0:
                break
        tile_of_node[node] = t
        tile_members[t].append(node)
        remaining[t] -= 1
        if remaining[t] > 0:
            heapq.heappush(heap, (s + int(deg[node]), t))

    perm = np.concatenate([np.asarray(m, dtype=np.int64) for m in tile_members])
    inv_perm = np.empty(n_nodes, dtype=np.int64)
    inv_perm[perm] = np.arange(n_nodes)

    tile_starts_nodes = np.concatenate([[0], np.cumsum(caps)])
    loc_of_node = inv_perm - tile_starts_nodes[tile_of_node]

    # --- half-balance refinement -------------------------------------------
    # Goal: for every target tile g, the number of its in-edges with source
    # table row < 25000 (cntA) and >= 25000 (cntB) are both <= 1024, so every
    # per-(tile,half) gather is exactly 8 chunks.  Nodes u (half A) and
    # v (half B) with equal in-degree swap tile assignments; this flips the
    # half of their out-edges and moves their in-edges between tiles g(u) and
    # g(v) (degree sums unchanged).
    half_tiles = n_tiles // 2
    cap = 1024
    rng = np.random.default_rng(12345)

    def halfA_of(node):
        return tile_of_node[node] < half_tiles

    # CSR of out-edges by source
    so = np.argsort(src, kind="stable")
    s_sorted = src[so]
    sptr = np.searchsorted(s_sorted, np.arange(n_nodes + 1))
    out_tgt = tgt[so]          # targets of node's out-edges (by src order)

    # CSR of in-edges by target
    to = np.argsort(tgt, kind="stable")
    t_sorted = tgt[to]
    tptr = np.searchsorted(t_sorted, np.arange(n_nodes + 1))
    in_src = src[to]

    hA = tile_of_node < half_tiles          # per node
    cnt = np.bincount(tile_of_node[tgt], weights=hA[src].astype(np.float64),
                      minlength=n_tiles).astype(np.int64)
    degsum = np.bincount(tile_of_node[tgt], minlength=n_tiles)

    # nodes by in-degree for pairing
    by_deg = {}
    for u in range(n_nodes):
        by_deg.setdefault(int(deg[u]), []).append(u)

    def excess():
        return (np.maximum(cnt - cap, 0)
                + np.maximum((degsum - cnt) - cap, 0))

    ex = excess()

    def swap_delta(u, v):
        """Total-excess delta from swapping tiles of u (A) and v (B)."""
        d = np.zeros(0)
        touched = {}

        def bump(g, dv):
            touched[g] = touched.get(g, 0) + dv
        gu, gv = tile_of_node[u], tile_of_node[v]
        for w in out_tgt[sptr[u]:sptr[u + 1]]:
            if w != u and w != v:
                bump(tile_of_node[w], -1)
        for w in out_tgt[sptr[v]:sptr[v + 1]]:
            if w != u and w != v:
                bump(tile_of_node[w], +1)
        inAu = int(hA[in_src[tptr[u]:tptr[u + 1]]].sum())
        inAv = int(hA[in_src[tptr[v]:tptr[v + 1]]].sum())
        bump(gu, -inAu + inAv)
        bump(gv, +inAu - inAv)
        # self/cross edges of u,v handled conservatively by recompute epochs
        delta = 0
        for g, dv in touched.items():
            c0 = cnt[g]
            c1 = c0 + dv
            e0 = max(c0 - cap, 0) + max((degsum[g] - c0) - cap, 0)
            e1 = max(c1 - cap, 0) + max((degsum[g] - c1) - cap, 0)
            delta += e1 - e0
        return delta, touched

    deadline = time.monotonic() + 35.0
    for _epoch in range(3):
        if time.monotonic() > deadline:
            break
        e_tile0 = tile_of_node[tgt]
        e_by_tile = np.argsort(e_tile0, kind="stable")
        tb = np.searchsorted(e_tile0[e_by_tile], np.arange(n_tiles + 1))
        tile_srcs = [src[e_by_tile[tb[g]:tb[g + 1]]] for g in range(n_tiles)]
        hA = tile_of_node < half_tiles
        cnt = np.bincount(tile_of_node[tgt],
                          weights=hA[src].astype(np.float64),
                          minlength=n_tiles).astype(np.int64)
        ex = excess()
        if ex.sum() == 0:
            break
        tries = 0
        best_ex = int(ex.sum())
        stall = 0
        while (ex.sum() > 0 and tries < 200000 and stall < 40000
               and time.monotonic() < deadline):
            tries += 1
            stall += 1
            cur = int(ex.sum())
            if cur < best_ex:
                best_ex = cur
                stall = 0
            bad = np.flatnonzero(ex)
            g = int(bad[int(rng.integers(len(bad)))])
            srcs = tile_srcs[g]
            over_a = cnt[g] > cap
            m = hA[srcs] if over_a else ~hA[srcs]
            cand = srcs[m]
            if len(cand) == 0:
                ex[g] = 0
                continue
            u = int(cand[int(rng.integers(len(cand)))])
            pool = by_deg.get(int(deg[u]), [])
            if len(pool) < 2:
                continue
            v = int(pool[int(rng.integers(len(pool)))])
            if hA[v] == hA[u]:
                continue
            if not hA[u]:
                u, v = v, u    # ensure u in A, v in B
            dlt, touched = swap_delta(u, v)
            if dlt < 0:
                gu, gv = tile_of_node[u], tile_of_node[v]
                tile_members[gu] = [v if x == u else x
                                    for x in tile_members[gu]]
                tile_members[gv] = [u if x == v else x
                                    for x in tile_members[gv]]
                tile_of_node[u], tile_of_node[v] = gv, gu
                hA[u] = not hA[u]
                hA[v] = not hA[v]
                for gg, dv in touched.items():
                    cnt[gg] += dv
                    ex[gg] = (max(cnt[gg] - cap, 0)
                              + max((degsum[gg] - cnt[gg]) - cap, 0))

    # rebuild perm/loc after refinement
    perm = np.concatenate([np.asarray(m, dtype=np.int64)
                           for m in tile_members])
    inv_perm = np.empty(n_nodes, dtype=np.int64)
    inv_perm[perm] = np.arange(n_nodes)
    loc_of_node = inv_perm - tile_starts_nodes[tile_of_node]

    e_tile = tile_of_node[tgt]
    e_order = np.argsort(e_tile, kind="stable")
    e_tile_sorted = e_tile[e_order]
    bounds = np.searchsorted(e_tile_sorted, np.arange(n_tiles + 1))

    # per-target inverse degree (cdeg); verify ew really is per-target const
    cinv = (1.0 / np.maximum(deg, 1.0)).astype(np.float32)
    assert np.allclose(ew, cinv[tgt], rtol=1e-4, atol=1e-7), \
        "edge weights are not inverse target degree"
    # cdeg per tile row [n_tiles, 128]
    cdeg = np.zeros((n_tiles, 128), np.float32)
    for t in range(n_tiles):
        mem = tile_members[t]
        cdeg[t, :len(mem)] = cinv[np.asarray(mem, dtype=np.int64)]

    return {
        "perm": perm,
        "lists": (e_order, bounds, inv_perm[src], loc_of_node[tgt]),
        "cdeg": cdeg,  # [n_tiles, 128]
    }


def plan_gather(plans, n_nodes, n_cores, T):
    """Merged-gather layout.

    For each (half h, tile t): ni[t,h] = roundup128(max_core max_branch count).
    Tiles are processed in groups of GRP; one dma_gather per (group, half).
    Returns per-branch per-core idx16 grids + meta (tgt cols f16) + cdeg.
    """
    half = n_nodes // 2
    nbr = len(plans)
    cnt = np.zeros((nbr, n_cores, T, 2), np.int64)
    data = {}
    for b, p in enumerate(plans):
        e_order, bounds, src_new, loc_tgt = p["lists"]
        for g in range(n_cores * T):
            c, t = g // T, g % T
            sl = e_order[bounds[g]:bounds[g + 1]]
            s = src_new[sl]
            for h in range(2):
                m = (s >= half) if h else (s < half)
                data[(b, c, t, h)] = (s[m] - h * half,
                                     loc_tgt[sl][m].astype(np.float32))
                cnt[b, c, t, h] = m.sum()
    ni = ((cnt.max(axis=(0, 1)) + 127) // 128) * 128   # [T, 2]
    ni = np.maximum(ni, 128)
    cf = ni // 128                                      # chunks per (t, h)
    cft = cf.sum(axis=1)                                # chunks per tile
    # group structure
    groups = [list(range(g0, min(g0 + GRP, T))) for g0 in range(0, T, GRP)]
    # idx / meta assembly
    C16 = int(ni.sum() // 16)
    idx16 = np.zeros((nbr, n_cores, 128, C16), np.int16)
    meta = np.full((nbr, n_cores, 128, int(cft.sum())), POISON, np.float32)
    for b in range(nbr):
        for c in range(n_cores):
            col = 0
            for grp in groups:
                for h in range(2):
                    for t in grp:
                        n = int(ni[t, h])
                        il, tl = data[(b, c, t, h)]
                        idx = np.zeros(n, np.int16)
                        idx[:len(il)] = il
                        tg = np.full(n, POISON, np.float32)
                        tg[:len(tl)] = tl
                        # slot (p, ch) = edge ch*128 + p
                        kcols = n // 16
                        slot = idx.reshape(-1, 128).T          # [128, cf]
                        grid = np.zeros((16, kcols), np.int16)
                        for k in range(kcols):
                            grid[:, k] = slot[16 * (k % 8):16 * (k % 8) + 16,
                                              k // 8]
                        idx16[b, c, :, col:col + kcols] = np.tile(grid, (8, 1))
                        col += kcols
                        mcol = int(cft[:t].sum())
                        moff = mcol + (int(cf[t, 0]) if h else 0)
                        meta[b, c, :, moff:moff + n // 128] = \
                            tg.reshape(-1, 128).T
    return {
        "ni": ni, "cf": cf, "cft": cft, "groups": groups,
        "idx16": idx16, "meta": meta.astype(np.float16),
    }


# ---------------------------------------------------------------------------
# Bass kernel builder
# ---------------------------------------------------------------------------

def build_gcn(n_cores, n_nodes, sizes, gp, dim=DIM):
    npc = n_nodes // n_cores
    T = len(sizes)
    TD = F16
    half = n_nodes // 2
    ni, cf, cft, groups = gp["ni"], gp["cf"], gp["cft"], gp["groups"]
    C16 = gp["idx16"].shape[3]
    MW = gp["meta"].shape[3]
    nc = bacc.Bacc("TRN2", target_bir_lowering=False, debug=False,
                   num_devices=n_cores)
    AT = mybir.ActivationFunctionType
    OP = mybir.AluOpType
    rg = [list(range(n_cores))]

    emb_in, idx_in, meta_in, cdeg_in, out_ext = {}, {}, {}, {}, {}
    for br in range(2):
        emb_in[br] = nc.dram_tensor(f"emb{br}", [npc, DPAD], F16,
                                    kind="ExternalInput")
        idx_in[br] = nc.dram_tensor(f"idx{br}", [128, C16], I16,
                                    kind="ExternalInput")
        meta_in[br] = nc.dram_tensor(f"meta{br}", [128, MW], F16,
                                     kind="ExternalInput")
        cdeg_in[br] = nc.dram_tensor(f"cdeg{br}", [128, T], F32,
                                     kind="ExternalInput")
        out_ext[br] = nc.dram_tensor(f"out{br}", [npc, dim], F32,
                                     kind="ExternalOutput")
    w_in = nc.dram_tensor("conv_w", [dim, dim], F32, kind="ExternalInput")
    b_in = nc.dram_tensor("conv_b", [128, dim], F32, kind="ExternalInput")
    iota_in = nc.dram_tensor("iota_h", [128, 128], F16, kind="ExternalInput")
    ident_in = nc.dram_tensor("ident", [128, 128], F32, kind="ExternalInput")

    row_slices = []
    off = 0
    for sz in sizes:
        row_slices.append((off, sz))
        off += sz

    with TileContext(nc) as tc:
        with (
            tc.tile_pool(name="const", bufs=1) as cpool,
            tc.tile_pool(name="dram", bufs=1, space="DRAM") as dpool,
            tc.tile_pool(name="work", bufs=3) as work,
            tc.tile_pool(name="gbuf", bufs=2) as gpool,
            tc.tile_pool(name="oh", bufs=8) as ohpool,
            tc.tile_pool(name="psum", bufs=2, space="PSUM") as pspool,
            tc.tile_pool(name="outs", bufs=3) as outp,
        ):
            # x0 bounce first so the head collective starts ASAP
            # into an Internal DRAM tile for the collective ([6250,256] f16 as
            # [125, 12800] flat chunks).
            x0_shard = {}
            for br in range(2):
                x0_shard[br] = dpool.tile([npc, DPAD], TD, name=f"x0s{br}")
                bt = work.tile([125, 12800], TD, tag="embbounce")
                nc.sync.dma_start(
                    bt[:], emb_in[br][:, :].rearrange(
                        "(a b) d -> a (b d)", a=125))
                nc.sync.dma_start(
                    x0_shard[br][:, :].rearrange("(a b) d -> a (b d)", a=125),
                    bt[:])

            # ---- constants ----
            w_a = cpool.tile([128, dim], F32)
            nc.sync.dma_start(w_a[:], w_in[0:128, :])
            w_b = cpool.tile([dim - 128, dim], F32)
            nc.sync.dma_start(w_b[:], w_in[128:dim, :])
            bb = cpool.tile([128, dim], F32)
            nc.sync.dma_start(bb[:], b_in[:, :])
            iota_h = cpool.tile([128, 128], F16)
            nc.sync.dma_start(iota_h[:], iota_in[:, :])
            ident = cpool.tile([128, 128], F32)
            nc.sync.dma_start(ident[:], ident_in[:, :])

            idx_sb, meta_sb, cdeg_sb = {}, {}, {}
            for br in range(2):
                idx_sb[br] = cpool.tile([128, C16], I16, name=f"idxsb{br}")
                nc.sync.dma_start(idx_sb[br][:], idx_in[br][:, :])
                meta_sb[br] = cpool.tile([128, MW], F16, name=f"metasb{br}")
                nc.sync.dma_start(meta_sb[br][:], meta_in[br][:, :])
                cdeg_sb[br] = cpool.tile([128, T], F32, name=f"cdegsb{br}")
                nc.sync.dma_start(cdeg_sb[br][:], cdeg_in[br][:, :])

            # ---- DRAM bounce/table tiles ----
            x0_tab, x1_shard, x1_tab = {}, {}, {}  # x0_shard built below
            for br in range(2):
                x0_tab[br] = dpool.tile([n_nodes, DPAD], TD,
                                        addr_space="Shared", name=f"x0t{br}")
                x1_shard[br] = dpool.tile([npc, DPAD], TD, name=f"x1s{br}")
                x1_tab[br] = dpool.tile([n_nodes, DPAD], TD,
                                        addr_space="Shared", name=f"x1t{br}")

            def allgather(shard, tab):
                nc.gpsimd.collective_compute(
                    "AllGather", mybir.AluOpType.bypass, replica_groups=rg,
                    ins=[shard], outs=[tab[:]])

            def layer(br, tab, dst, out_dt, wide, interject=None):
                """Process one (branch, layer). interject: callback emitted
                after the first gather group (to overlap collectives)."""
                tabv = tab[:]
                for gi, grp in enumerate(groups):
                    # merged gathers for this group, one per half
                    cols_a = int(cf[grp, 0].sum())
                    cols_b = int(cf[grp, 1].sum())
                    na, nb = cols_a * 128, cols_b * 128
                    c16off = int(ni[:grp[0]].sum() // 16)
                    ga = gpool.tile([128, cols_a, DPAD], TD, tag="GA")
                    gb = gpool.tile([128, cols_b, DPAD], TD, tag="GB")
                    # idx cols for this group: per tile the a-cols then b-cols
                    # were packed contiguously; a and b interleave per tile, so
                    # gather per tile-half run instead: emit one gather per
                    # half spanning the group by using a gathered idx layout
                    # that was packed group-contiguously in plan_gather.
                    nc.gpsimd.dma_gather(
                        ga[:], tabv[0:half, :],
                        idx_sb[br][:, c16off:c16off + na // 16],
                        na, na, DPAD, single_packet=False)
                    nc.gpsimd.dma_gather(
                        gb[:], tabv[half:n_nodes, :],
                        idx_sb[br][:, c16off + na // 16:
                                   c16off + (na + nb) // 16],
                        nb, nb, DPAD, single_packet=False)
                    if gi == 0 and interject is not None:
                        interject()
                    ca = 0
                    cb = 0
                    for t in grp:
                        off, sz = row_slices[t]
                        tb = int(cft[:t].sum())
                        cfa, cfb = int(cf[t, 0]), int(cf[t, 1])
                        u = pspool.tile([128, dim], F32, tag="u")
                        ms = meta_sb[br]
                        for c in range(cfa + cfb):
                            rhs = (ga[:, ca + c, 0:dim] if c < cfa
                                   else gb[:, cb + c - cfa, 0:dim])
                            oh = ohpool.tile([128, 128], F16, tag="oh")
                            tcol = ms[:, tb + c:tb + c + 1]
                            nc.vector.tensor_tensor(
                                oh[:], iota_h[:],
                                tcol.to_broadcast((128, 128)),
                                op=mybir.AluOpType.is_equal)
                            nc.tensor.matmul(u[:], lhsT=oh[:], rhs=rhs,
                                             start=(c == 0),
                                             stop=(c == cfa + cfb - 1))
                        ca += cfa
                        cb += cfb
                        # evacuate PSUM with fused 1/deg scale
                        u_s = work.tile([128, dim], F32, tag="u_s")
                        cd = cdeg_sb[br][:, t:t + 1]
                        nc.vector.tensor_tensor(
                            u_s[:], u[:], cd.to_broadcast((128, dim)),
                            op=mybir.AluOpType.mult)
                        ut = pspool.tile([128, 256], F32, tag="uT")
                        nc.tensor.transpose(ut[:, 0:128], u_s[:, 0:128],
                                            ident[:])
                        nc.tensor.transpose(ut[0:dim - 128, 128:256],
                                            u_s[:, 128:dim], ident[:])
                        ut_s = work.tile([128, 256], F32, tag="uT_s")
                        nc.vector.tensor_copy(ut_s[:, 0:128], ut[:, 0:128])
                        nc.vector.tensor_copy(ut_s[0:dim - 128, 128:256],
                                              ut[0:dim - 128, 128:256])
                        v = pspool.tile([128, dim], F32, tag="v")
                        nc.tensor.matmul(v[:], lhsT=ut_s[:, 0:128], rhs=w_a[:],
                                         start=True, stop=False)
                        nc.tensor.matmul(v[:], lhsT=ut_s[0:dim - 128, 128:256],
                                         rhs=w_b[:], start=False, stop=True)
                        xadd = outp.tile([128, dim], F32, tag="xadd")
                        nc.vector.tensor_tensor(xadd[:], v[:], bb[:],
                                                op=mybir.AluOpType.add)
                        if wide:
                            xo = outp.tile([128, dim], out_dt, tag="xo")
                            nc.scalar.activation(xo[:], xadd[:], AT.Relu)
                            nc.sync.dma_start(dst[off:off + sz, 0:dim],
                                              xo[:sz, :])
                        else:
                            xo = outp.tile([128, dim], out_dt, tag="xo")
                            nc.scalar.activation(xo[:], xadd[:], AT.Relu)
                            nc.sync.dma_start(dst[off:off + sz, :], xo[:sz])

            # schedule: head
            allgather(x0_shard[0][:], x0_tab[0])
            # L1 branch 0; allgather(x0 br1) right after the first gathers
            layer(0, x0_tab[0], x1_shard[0], TD, wide=True,
                  interject=lambda: allgather(x0_shard[1][:], x0_tab[1]))
            allgather(x1_shard[0][:], x1_tab[0])
            layer(1, x0_tab[1], x1_shard[1], TD, wide=True)
            allgather(x1_shard[1][:], x1_tab[1])
            layer(0, x1_tab[0], out_ext[0], F32, wide=False)
            layer(1, x1_tab[1], out_ext[1], F32, wide=False)

    nc.compile()
    return nc


# ---------------------------------------------------------------------------
# Entry point
# ---------------------------------------------------------------------------

def _run(match_emb, ref_emb, conv_w, conv_b, match_edges, ref_edges,
         match_ew, ref_ew, n_nodes, n_cores, trace=False):
    npc = n_nodes // n_cores
    sizes = tile_sizes_for(npc)
    plans = [plan_branch(e, w, n_nodes, n_cores, sizes)
             for e, w in ((match_edges, match_ew), (ref_edges, ref_ew))]
    gp = plan_gather(plans, n_nodes, n_cores, len(sizes))

    nc = build_gcn(n_cores, n_nodes, sizes, gp)

    embs = []
    for e in (match_emb, ref_emb):
        x = np.asarray(e, np.float32)
        x = x / np.maximum(np.linalg.norm(x, axis=1, keepdims=True), 1e-12)
        xp = np.zeros((x.shape[0], DPAD), np.float16)
        xp[:, :DIM] = x.astype(np.float16)
        embs.append(xp)
    emb_perm = [embs[b][plans[b]["perm"]] for b in range(2)]
    b_bcast = np.ascontiguousarray(
        np.broadcast_to(np.asarray(conv_b, np.float32)[None, :], (128, DIM)))
    w_np = np.ascontiguousarray(np.asarray(conv_w, np.float32))
    T = len(sizes)

    in_maps = []
    for c in range(n_cores):
        iota_np = np.broadcast_to(np.arange(128, dtype=np.float16)[None, :],
                                  (128, 128)).copy()
        ident_np = np.eye(128, dtype=np.float32)
        m = {"conv_w": w_np, "conv_b": b_bcast,
             "iota_h": iota_np, "ident": ident_np}
        for br in range(2):
            m[f"emb{br}"] = np.ascontiguousarray(
                emb_perm[br][c * npc:(c + 1) * npc])
            m[f"idx{br}"] = np.ascontiguousarray(gp["idx16"][br, c])
            m[f"meta{br}"] = np.ascontiguousarray(gp["meta"][br, c])
            # cdeg [n_tiles, 128] -> this core's T tiles as [128, T]
            m[f"cdeg{br}"] = np.ascontiguousarray(
                plans[br]["cdeg"][c * T:(c + 1) * T].T)
        in_maps.append(m)

    res = bass_utils.run_bass_kernel_spmd(
        nc, in_maps, core_ids=list(range(n_cores)), trace=trace)

    outs = []
    for br in range(2):
        full = np.empty((n_nodes, DIM), dtype=np.float32)
        perm = plans[br]["perm"]
        for c in range(n_cores):
            full[perm[c * npc:(c + 1) * npc]] = res.results[c][f"out{br}"]
        outs.append(full)
    return (outs[0], outs[1]), res


def kernel(match_emb, ref_emb, conv_w, conv_b, match_edges, ref_edges,
           match_ew, ref_ew):
    trace = bool(int(os.environ.get("KERNEL_TRACE", "0")))
    (out_m, out_r), _ = _run(match_emb, ref_emb, conv_w, conv_b,
                             match_edges, ref_edges, match_ew, ref_ew,
                             N_NODES, N_CORES, trace=trace)
    return out_m, out_r
